# revision 13
# baseline (speedup 1.0000x reference)
"""Trainium2 Bass kernel for nn_Ada_PoLIN (InstanceNorm+LayerNorm -> concat ->
1x1 conv -> per-sample scale/shift).

Math: for sample b,
  IN = (x - mu_in) * r_in            (per-channel spatial stats)
  LN = (x - mu_ln) * r_ln            (per-sample stats)
  c  = W1 @ IN + W2 @ LN             (W = [W1 | W2], 1x1 conv)
  out = gamma * c + beta

This collapses to a single per-sample channel-mixing matmul:
  out[o, s] = gamma[o] * ( sum_i A[o,i] * x[i,s] + bias[o] ) + beta[o]
  A[o, i]   = W1[o,i] * r_in[i] + r_ln * W2[o,i]
  bias[o]   = -sum_i W1[o,i]*r_in[i]*mu_in[i] - r_ln*mu_ln*sum_i W2[o,i]

Sharding: data-parallel over batch, one sample per NeuronCore (B=8, 8 cores).
No cross-core communication. Per core: one pass over x for stats (bn_stats,
overlapped with DMA-in), build A^T (tiny), then a [256,256]x[256,16384]
matmul streamed through PSUM with the gamma/beta epilogue fused into the
PSUM->SBUF evacuation, and chunked DMA-out.
"""

import sys

if "/opt/trn_rl_repo" not in sys.path:
    sys.path.insert(0, "/opt/trn_rl_repo")

from contextlib import ExitStack

import numpy as np

import concourse.bacc as bacc
import concourse.tile as tile
from concourse import mybir
from concourse.bass_utils import run_bass_kernel_spmd
from concourse.masks import make_identity

B, C, H, W_SP = 8, 256, 128, 128
HW = H * W_SP            # 16384 spatial elements
TWO_C = 2 * C
N_CORES = 8
EPS = 1e-5
P = 128                  # partitions
KT = C // P              # 2 contraction (input-channel) tiles
MT = C // P              # 2 output-channel tiles
CHUNK = 2048             # spatial chunk per x tile / DMA
NCH = HW // CHUNK        # 8 chunks per k-tile
NSUB = CHUNK // 512      # bn_stats subgroups per chunk
NQ = 512                 # matmul free-dim chunk (one PSUM bank)
QPC = CHUNK // NQ        # matmul chunks per stage tile

USE_F32R = True          # float32r matmul: full-rate fp32 path on TensorE

F32 = mybir.dt.float32
F32R = mybir.dt.float32r


def build(use_f32r: bool = USE_F32R):
    nc = bacc.Bacc("TRN2", num_devices=N_CORES)
    x_ext = nc.declare_dram_parameter("x", [C, HW], F32, isOutput=False)
    p_ext = nc.declare_dram_parameter("params", [TWO_C], F32, isOutput=False)
    w_ext = nc.declare_dram_parameter("W", [C, TWO_C], F32, isOutput=False)
    out_ext = nc.declare_dram_parameter("out", [C, HW], F32, isOutput=True)

    x_r = x_ext.ap().rearrange("(t p) s -> t p s", p=P)      # [KT, 128, HW]
    out_r = out_ext.ap().rearrange("(t p) s -> t p s", p=P)  # [MT, 128, HW]
    p_r = p_ext.ap().rearrange("(g p) -> g p", p=P)          # [4, 128]
    w_r = w_ext.ap().rearrange("(t p) i -> t p i", p=P)      # [MT, 128, 2C]

    mm_dt = F32R if use_f32r else F32

    with tile.TileContext(nc) as tc, ExitStack() as ctx:
        xpool = ctx.enter_context(tc.tile_pool(name="x", bufs=1))
        wpool = ctx.enter_context(tc.tile_pool(name="w", bufs=1))
        small = ctx.enter_context(tc.tile_pool(name="small", bufs=1))
        stage = ctx.enter_context(tc.tile_pool(name="stage", bufs=3))
        psum_mm = ctx.enter_context(
            tc.tile_pool(name="psum_mm", bufs=6, space="PSUM")
        )
        psum_su = ctx.enter_context(
            tc.tile_pool(name="psum_su", bufs=2, space="PSUM")
        )

        # ---- constants / weights (ACT-ring DMAs, emitted first so the
        # PE transposes + ACT copies clear before stats work floods ACT) ----
        ident = small.tile([P, P], F32, tag="ident")
        make_identity(nc, ident)
        ones = small.tile([P, P], F32, tag="ones")
        nc.vector.memset(ones, 1.0)
        epst = small.tile([P, 1], F32, tag="eps")
        nc.vector.memset(epst, EPS)

        w_sb = [wpool.tile([P, TWO_C], F32, tag=f"wsb{m}", name=f"wsb{m}") for m in range(MT)]
        for m in range(MT):
            nc.sync.dma_start(out=w_sb[m], in_=w_r[m])
        pg = small.tile([4, P], F32, tag="pg")
        nc.sync.dma_start(out=pg, in_=p_r)

        # params transpose: [4,128] -> [128,4] = [gamma0|gamma1|beta0|beta1]
        pt_ps = psum_su.tile([P, 4], F32, tag="setup")
        nc.tensor.transpose(pt_ps, pg, ident[:4, :4])
        pb = small.tile([P, 4], F32, tag="pb")
        nc.scalar.copy(out=pb, in_=pt_ps)

        # W1T/W2T: [i_part, o_free] tiles via PE transpose of 128x128 blocks
        w1t = [small.tile([P, C], F32, tag=f"w1t{k}", name=f"w1t{k}") for k in range(KT)]
        w2t = [small.tile([P, C], F32, tag=f"w2t{k}", name=f"w2t{k}") for k in range(KT)]
        for k in range(KT):
            for m in range(MT):
                ps = psum_su.tile([P, P], F32, tag="setup")
                nc.tensor.transpose(
                    ps, w_sb[m][:, k * P : (k + 1) * P], ident
                )
                nc.scalar.copy(out=w1t[k][:, m * P : (m + 1) * P], in_=ps)
                ps2 = psum_su.tile([P, P], F32, tag="setup")
                nc.tensor.transpose(
                    ps2, w_sb[m][:, C + k * P : C + (k + 1) * P], ident
                )
                nc.scalar.copy(out=w2t[k][:, m * P : (m + 1) * P], in_=ps2)

        # ---- x load + one-pass stats: bn_stats on DVE for most chunks;
        # chunk c==ACT_C goes to ACT (sum + sumsq accum passes) to keep the
        # DVE backlog shorter than the DMA window; the last chunk is DMA'd
        # in two halves so its stats clear quickly after arrival ----
        ACT_C = 6
        DVE_N = (NCH - 1) * NSUB  # bn_stats subgroup slots per k
        xt = [[None] * NCH for _ in range(KT)]
        st = [small.tile([P, DVE_N, 6], F32, tag=f"st{k}", name=f"st{k}") for k in range(KT)]
        asum = small.tile([P, KT, 2], F32, tag="asum")  # [sum, sumsq] per k for ACT_C
        slot = [0] * KT
        for c in range(NCH):
            for k in range(KT):
                t = xpool.tile([P, CHUNK], mm_dt, tag=f"x{k}_{c}", name=f"x{k}_{c}")
                xt[k][c] = t
                src_ap = x_r[k, :, c * CHUNK : (c + 1) * CHUNK]
                if use_f32r:
                    src_ap = src_ap.bitcast(mm_dt)
                if c == NCH - 1:
                    half = CHUNK // 2
                    nc.sync.dma_start(out=t[:, :half], in_=src_ap[:, :half])
                    nc.sync.dma_start(out=t[:, half:], in_=src_ap[:, half:])
                else:
                    nc.sync.dma_start(out=t, in_=src_ap)
                tf = t.bitcast(F32)
                tv = tf.rearrange("p (a b) -> p a b", b=512)
                if c == ACT_C:
                    ascr = stage.tile([P, CHUNK], F32, tag="ascr", name="ascr", bufs=2)
                    nc.scalar.activation(
                        out=ascr, in_=tf,
                        func=mybir.ActivationFunctionType.Identity,
                        accum_out=asum[:, k, 0:1],
                    )
                    ascr2 = stage.tile([P, CHUNK], F32, tag="ascr", name="ascr2", bufs=2)
                    nc.scalar.activation(
                        out=ascr2, in_=tf,
                        func=mybir.ActivationFunctionType.Square,
                        accum_out=asum[:, k, 1:2],
                    )
                else:
                    for j in range(NSUB):
                        nc.vector.bn_stats(
                            out=st[k][:, slot[k], :], in_=tv[:, j, :]
                        )
                        slot[k] += 1
                if k == 0 and c >= 2:
                    # keep the PE clock-gate warm through the load phase:
                    # tiny matmul paced by each chunk arrival, result unused
                    wps = psum_su.tile([P, 2], F32, tag="setup", name=f"warm{c}")
                    nc.tensor.matmul(
                        wps, ones, tf[:, 0:2], start=True, stop=True
                    )
        assert slot[0] == DVE_N and slot[1] == DVE_N

        # ---- finalize stats ----
        mv = [small.tile([P, 2], F32, tag=f"mv{k}", name=f"mv{k}") for k in range(KT)]
        rin = [small.tile([P, 1], F32, tag=f"rin{k}", name=f"rin{k}") for k in range(KT)]
        tk = [small.tile([P, 2], F32, tag=f"tk{k}", name=f"tk{k}") for k in range(KT)]
        vk = [small.tile([P, 1], F32, tag=f"vk{k}", name=f"vk{k}") for k in range(KT)]
        SUB_N = float(DVE_N * 512)   # elements covered by bn_stats per k
        var = [small.tile([P, 1], F32, tag=f"var{k}", name=f"var{k}") for k in range(KT)]
        for k in range(KT):
            nc.vector.bn_aggr(out=mv[k], in_=st[k])
            mu_s = mv[k][:, 0:1]
            var_s = mv[k][:, 1:2]
            # combined sums: S = mu_s*N_s + act_sum ; Q = (var_s+mu_s^2)*N_s + act_sq
            nc.vector.scalar_tensor_tensor(
                out=tk[k][:, 0:1], in0=mu_s, scalar=SUB_N,
                in1=asum[:, k, 0:1], op0=mybir.AluOpType.mult,
                op1=mybir.AluOpType.add,
            )
            nc.vector.scalar_tensor_tensor(
                out=tk[k][:, 1:2], in0=mu_s, scalar=mu_s, in1=var_s,
                op0=mybir.AluOpType.mult, op1=mybir.AluOpType.add,
            )
            nc.vector.scalar_tensor_tensor(
                out=tk[k][:, 1:2], in0=tk[k][:, 1:2], scalar=SUB_N,
                in1=asum[:, k, 1:2], op0=mybir.AluOpType.mult,
                op1=mybir.AluOpType.add,
            )
            # tk = [mu, E[x^2]] for the full HW extent
            nc.vector.tensor_scalar_mul(out=tk[k], in0=tk[k], scalar1=1.0 / HW)
            # var = E[x^2] - mu^2 ; r_in = 1/sqrt(var+eps)
            nc.vector.tensor_mul(out=var[k], in0=tk[k][:, 0:1], in1=tk[k][:, 0:1])
            nc.vector.tensor_sub(out=var[k], in0=tk[k][:, 1:2], in1=var[k])
            nc.scalar.activation(
                out=rin[k], in_=var[k],
                func=mybir.ActivationFunctionType.Abs_reciprocal_sqrt,
                bias=epst, scale=1.0,
            )

        # LN sums replicated on all partitions: ones^T @ t
        ln_ps = psum_su.tile([P, 2], F32, tag="setup")
        for k in range(KT):
            nc.tensor.matmul(
                ln_ps, ones, tk[k], start=(k == 0), stop=(k == KT - 1)
            )
        mu_ln = small.tile([P, 1], F32, tag="mu_ln")
        m2_ln = small.tile([P, 1], F32, tag="m2_ln")
        var_ln = small.tile([P, 1], F32, tag="var_ln")
        rln = small.tile([P, 1], F32, tag="rln")
        w2s = small.tile([P, 1], F32, tag="w2s")
        nc.scalar.mul(out=mu_ln, in_=ln_ps[:, 0:1], mul=1.0 / C)
        nc.scalar.mul(out=m2_ln, in_=ln_ps[:, 1:2], mul=1.0 / C)
        # var_ln = m2 - mu^2
        nc.vector.tensor_mul(out=var_ln, in0=mu_ln, in1=mu_ln)
        nc.vector.tensor_sub(out=var_ln, in0=m2_ln, in1=var_ln)
        nc.scalar.activation(
            out=rln, in_=var_ln,
            func=mybir.ActivationFunctionType.Abs_reciprocal_sqrt,
            bias=epst, scale=1.0,
        )
        # w2s = -(r_ln * mu_ln)
        nc.vector.scalar_tensor_tensor(
            out=w2s, in0=rln, scalar=-1.0, in1=mu_ln,
            op0=mybir.AluOpType.mult, op1=mybir.AluOpType.mult,
        )
        # v_k = -(r_in * mu_in)
        for k in range(KT):
            nc.vector.scalar_tensor_tensor(
                out=vk[k], in0=rin[k], scalar=-1.0, in1=tk[k][:, 0:1],
                op0=mybir.AluOpType.mult, op1=mybir.AluOpType.mult,
            )

        # ---- A^T tiles: AT_k[i, o] = W1T*r_in[i] + r_ln*W2T ----
        at = [small.tile([P, C], mm_dt, tag=f"at{k}", name=f"at{k}") for k in range(KT)]
        for k in range(KT):
            tmp = small.tile([P, C], F32, tag=f"attmp{k}", name=f"attmp{k}")
            nc.vector.tensor_scalar_mul(out=tmp, in0=w2t[k], scalar1=rln)
            nc.vector.scalar_tensor_tensor(
                out=at[k], in0=w1t[k], scalar=rin[k], in1=tmp,
                op0=mybir.AluOpType.mult, op1=mybir.AluOpType.add,
            )

        # ---- bias and epilogue scalars per m ----
        gs = [pb[:, m : m + 1] for m in range(MT)]          # gamma_m
        bt = [pb[:, MT + m : MT + m + 1] for m in range(MT)]  # beta_m
        bs = [small.tile([P, 1], F32, tag=f"bs{m}", name=f"bs{m}") for m in range(MT)]
        for m in range(MT):
            bps = psum_su.tile([P, 1], F32, tag="setup")
            msl = slice(m * P, (m + 1) * P)
            nc.tensor.matmul(bps, w1t[0][:, msl], vk[0], start=True, stop=False)
            nc.tensor.matmul(bps, w1t[1][:, msl], vk[1], start=False, stop=False)
            nc.tensor.matmul(bps, w2t[0][:, msl], w2s, start=False, stop=False)
            nc.tensor.matmul(bps, w2t[1][:, msl], w2s, start=False, stop=True)
            # bs = gamma * bias + beta
            nc.scalar.activation(
                out=bs[m], in_=bps,
                func=mybir.ActivationFunctionType.Identity,
                scale=gs[m], bias=bt[m],
            )

        # ---- main matmul + fused epilogue + chunked DMA out ----
        at_mm = at
        for nb in range(NCH):
            for m in range(MT):
                stg = stage.tile([P, CHUNK], F32, tag=f"stage{m}", name=f"stage{m}")
                msl = slice(m * P, (m + 1) * P)
                for q in range(QPC):
                    ps = psum_mm.tile([P, NQ], F32)
                    qsl = slice(q * NQ, (q + 1) * NQ)
                    for k in range(KT):
                        rhs = xt[k][nb][:, qsl]
                        nc.tensor.matmul(
                            ps, at_mm[k][:, msl], rhs,
                            start=(k == 0), stop=(k == KT - 1),
                        )
                    # epilogue: out = gamma*psum + (gamma*bias+beta)
                    if (nb * MT + m) % 2 == 0:
                        nc.scalar.activation(
                            out=stg[:, qsl], in_=ps,
                            func=mybir.ActivationFunctionType.Identity,
                            bias=bs[m], scale=gs[m],
                        )
                    else:
                        nc.vector.tensor_scalar(
                            out=stg[:, qsl], in0=ps, scalar1=gs[m],
                            scalar2=bs[m], op0=mybir.AluOpType.mult,
                            op1=mybir.AluOpType.add,
                        )
                if nb == 0:
                    for q in range(QPC):
                        nc.sync.dma_start(
                            out=out_r[m, :, nb * CHUNK + q * NQ : nb * CHUNK + (q + 1) * NQ],
                            in_=stg[:, q * NQ : (q + 1) * NQ],
                        )
                else:
                    nc.sync.dma_start(
                        out=out_r[m, :, nb * CHUNK : (nb + 1) * CHUNK], in_=stg
                    )

    nc.compile()
    return nc


_built = {}


def _get(use_f32r: bool = USE_F32R):
    if use_f32r not in _built:
        _built[use_f32r] = build(use_f32r)
    return _built[use_f32r]


def run(x, params, W, trace=False, use_f32r=USE_F32R, **kw):
    nc = _get(use_f32r)
    x = np.ascontiguousarray(np.asarray(x, dtype=np.float32))
    params = np.ascontiguousarray(np.asarray(params, dtype=np.float32))
    W = np.ascontiguousarray(np.asarray(W, dtype=np.float32))
    in_maps = [
        {
            "x": x[b].reshape(C, HW),
            "params": params[b],
            "W": W,
        }
        for b in range(B)
    ]
    res = run_bass_kernel_spmd(
        nc, in_maps, list(range(N_CORES)), trace=trace, **kw
    )
    out = np.stack(
        [res.results[b]["out"].reshape(C, H, W_SP) for b in range(B)]
    ).astype(np.float32)
    return out, res


def kernel(x, params, W):
    out, _ = run(x, params, W)
    return out


# revision 26
# speedup vs baseline: 1.0527x; 1.0527x over previous
"""Trainium2 Bass kernel for nn_Ada_PoLIN (InstanceNorm+LayerNorm -> concat ->
1x1 conv -> per-sample scale/shift).

Math: for sample b,
  IN = (x - mu_in) * r_in            (per-channel spatial stats)
  LN = (x - mu_ln) * r_ln            (per-sample stats)
  c  = W1 @ IN + W2 @ LN             (W = [W1 | W2], 1x1 conv)
  out = gamma * c + beta

This collapses to a single per-sample channel-mixing matmul:
  out[o, s] = gamma[o] * ( sum_i A[o,i] * x[i,s] + bias[o] ) + beta[o]
  A[o, i]   = W1[o,i] * r_in[i] + r_ln * W2[o,i]
  bias[o]   = -sum_i W1[o,i]*r_in[i]*mu_in[i] - r_ln*mu_ln*sum_i W2[o,i]

Sharding: data-parallel over batch, one sample per NeuronCore (B=8, 8 cores).
No cross-core communication. Per core: one pass over x for stats (bn_stats,
overlapped with DMA-in), build A^T (tiny), then a [256,256]x[256,16384]
matmul streamed through PSUM with the gamma/beta epilogue fused into the
PSUM->SBUF evacuation, and chunked DMA-out.
"""

import sys

if "/opt/trn_rl_repo" not in sys.path:
    sys.path.insert(0, "/opt/trn_rl_repo")

from contextlib import ExitStack

import numpy as np

import concourse.bacc as bacc
import concourse.tile as tile
from concourse import mybir
from concourse.bass_utils import run_bass_kernel_spmd
from concourse.masks import make_identity

B, C, H, W_SP = 8, 256, 128, 128
HW = H * W_SP            # 16384 spatial elements
TWO_C = 2 * C
N_CORES = 8
EPS = 1e-5
P = 128                  # partitions
KT = C // P              # 2 contraction (input-channel) tiles
MT = C // P              # 2 output-channel tiles
CHUNK = 2048             # spatial chunk per x tile / DMA
NCH = HW // CHUNK        # 8 chunks per k-tile
NSUB = CHUNK // 512      # bn_stats subgroups per chunk
NQ = 512                 # matmul free-dim chunk (one PSUM bank)
QPC = CHUNK // NQ        # matmul chunks per stage tile

USE_F32R = True          # float32r matmul: full-rate fp32 path on TensorE

F32 = mybir.dt.float32
F32R = mybir.dt.float32r


def build(use_f32r: bool = USE_F32R):
    nc = bacc.Bacc("TRN2", num_devices=N_CORES)
    x_ext = nc.declare_dram_parameter("x", [C, HW], F32, isOutput=False)
    p_ext = nc.declare_dram_parameter("params", [TWO_C], F32, isOutput=False)
    w_ext = nc.declare_dram_parameter("W", [C, TWO_C], F32, isOutput=False)
    out_ext = nc.declare_dram_parameter("out", [C, HW], F32, isOutput=True)

    x_r = x_ext.ap().rearrange("(t p) s -> t p s", p=P)      # [KT, 128, HW]
    out_r = out_ext.ap().rearrange("(t p) s -> t p s", p=P)  # [MT, 128, HW]
    p_r = p_ext.ap().rearrange("(g p) -> g p", p=P)          # [4, 128]
    w_r = w_ext.ap().rearrange("(t p) i -> t p i", p=P)      # [MT, 128, 2C]

    mm_dt = F32R if use_f32r else F32

    with tile.TileContext(nc) as tc, ExitStack() as ctx:
        xpool = ctx.enter_context(tc.tile_pool(name="x", bufs=1))
        wpool = ctx.enter_context(tc.tile_pool(name="w", bufs=1))
        small = ctx.enter_context(tc.tile_pool(name="small", bufs=1))
        stage = ctx.enter_context(tc.tile_pool(name="stage", bufs=4))
        psum_mm = ctx.enter_context(
            tc.tile_pool(name="psum_mm", bufs=6, space="PSUM")
        )
        psum_su = ctx.enter_context(
            tc.tile_pool(name="psum_su", bufs=2, space="PSUM")
        )

        # ---- constants / weights (ACT-ring DMAs, emitted first so the
        # PE transposes + ACT copies clear before stats work floods ACT) ----
        ident = small.tile([P, P], F32, tag="ident")
        make_identity(nc, ident)
        ones = small.tile([P, P], F32, tag="ones")
        nc.vector.memset(ones, 1.0)
        epst = small.tile([P, 1], F32, tag="eps")
        nc.vector.memset(epst, EPS)

        w_sb = [wpool.tile([P, TWO_C], F32, tag=f"wsb{m}", name=f"wsb{m}") for m in range(MT)]
        pg = small.tile([4, P], F32, tag="pg")

        # params transpose: [4,128] -> [128,4] = [gamma0|gamma1|beta0|beta1]
        pt_ps = psum_su.tile([P, 4], F32, tag="setup")
        nc.tensor.transpose(pt_ps, pg, ident[:4, :4])
        pb = small.tile([P, 4], F32, tag="pb")
        nc.scalar.copy(out=pb, in_=pt_ps)

        # W1T/W2T: [i_part, o_free] tiles via PE transpose of 128x128 blocks
        w1t = [small.tile([P, C], F32, tag=f"w1t{k}", name=f"w1t{k}") for k in range(KT)]
        w2t = [small.tile([P, C], F32, tag=f"w2t{k}", name=f"w2t{k}") for k in range(KT)]
        for k in range(KT):
            for m in range(MT):
                ps = psum_su.tile([P, P], F32, tag="setup")
                nc.tensor.transpose(
                    ps, w_sb[m][:, k * P : (k + 1) * P], ident
                )
                nc.scalar.copy(out=w1t[k][:, m * P : (m + 1) * P], in_=ps)
                ps2 = psum_su.tile([P, P], F32, tag="setup")
                nc.tensor.transpose(
                    ps2, w_sb[m][:, C + k * P : C + (k + 1) * P], ident
                )
                nc.scalar.copy(out=w2t[k][:, m * P : (m + 1) * P], in_=ps2)

        # ---- x load + one-pass stats: bn_stats on DVE for most chunks;
        # chunk c==ACT_C goes to ACT (sum + sumsq accum passes) to keep the
        # DVE backlog shorter than the DMA window; the last chunk is DMA'd
        # in two halves so its stats clear quickly after arrival ----
        ACT_C = -1
        DVE_N = NCH * NSUB  # bn_stats subgroup slots per k
        xt = [[None] * NCH for _ in range(KT)]
        st = [small.tile([P, DVE_N, 6], F32, tag=f"st{k}", name=f"st{k}") for k in range(KT)]
        slot = [0] * KT
        for c in range(NCH):
            for k in range(KT):
                t = xpool.tile([P, CHUNK], mm_dt, tag=f"x{k}_{c}", name=f"x{k}_{c}")
                xt[k][c] = t
                src_ap = x_r[k, :, c * CHUNK : (c + 1) * CHUNK]
                if use_f32r:
                    src_ap = src_ap.bitcast(mm_dt)
                if c == NCH - 1:
                    half = CHUNK // 2
                    nc.sync.dma_start(out=t[:, :half], in_=src_ap[:, :half])
                    nc.sync.dma_start(out=t[:, half:], in_=src_ap[:, half:])
                else:
                    nc.sync.dma_start(out=t, in_=src_ap)
                tf = t.bitcast(F32)
                tv = tf.rearrange("p (a b) -> p a b", b=512)
                for j in range(NSUB):
                    nc.vector.bn_stats(
                        out=st[k][:, slot[k], :], in_=tv[:, j, :]
                    )
                    slot[k] += 1
            if c == 0:
                # weights/params queue right behind the first x chunk
                for m in range(MT):
                    nc.sync.dma_start(out=w_sb[m], in_=w_r[m])
                nc.sync.dma_start(out=pg, in_=p_r)
            if c >= NCH - 2:
                # dense warm-up matmuls on the last chunk arrivals: bring the
                # PE clock-gate to 8/8 right before the real matmuls start
                for k in range(KT):
                    for q in range(2):
                        wps = psum_su.tile(
                            [P, NQ], F32, tag="setup", name=f"warm{c}_{k}_{q}"
                        )
                        nc.tensor.matmul(
                            wps, w1t[0][:, 0:P],
                            xt[k][c].bitcast(F32)[:, q * NQ : (q + 1) * NQ],
                            start=True, stop=True,
                        )

        assert slot[0] == DVE_N and slot[1] == DVE_N

        # ---- finalize stats ----
        mv = [small.tile([P, 2], F32, tag=f"mv{k}", name=f"mv{k}") for k in range(KT)]
        rin = [small.tile([P, 1], F32, tag=f"rin{k}", name=f"rin{k}") for k in range(KT)]
        tk = [small.tile([P, 2], F32, tag=f"tk{k}", name=f"tk{k}") for k in range(KT)]
        vk = [small.tile([P, 1], F32, tag=f"vk{k}", name=f"vk{k}") for k in range(KT)]
        for k in range(KT):
            nc.vector.bn_aggr(out=mv[k], in_=st[k])
            mu_k = mv[k][:, 0:1]
            var_k = mv[k][:, 1:2]
            # r_in = 1/sqrt(var+eps)
            nc.scalar.activation(
                out=rin[k], in_=var_k,
                func=mybir.ActivationFunctionType.Abs_reciprocal_sqrt,
                bias=epst, scale=1.0,
            )
            # tk = [mu, E[x^2]] for the LN cross-channel sums
            nc.vector.tensor_copy(out=tk[k][:, 0:1], in_=mu_k)
            nc.vector.scalar_tensor_tensor(
                out=tk[k][:, 1:2], in0=mu_k, scalar=mu_k, in1=var_k,
                op0=mybir.AluOpType.mult, op1=mybir.AluOpType.add,
            )

        # LN sums replicated on all partitions: ones^T @ t
        ln_ps = psum_su.tile([P, 2], F32, tag="setup")
        for k in range(KT):
            nc.tensor.matmul(
                ln_ps, ones, tk[k], start=(k == 0), stop=(k == KT - 1)
            )
        mu_ln = small.tile([P, 1], F32, tag="mu_ln")
        m2_ln = small.tile([P, 1], F32, tag="m2_ln")
        var_ln = small.tile([P, 1], F32, tag="var_ln")
        rln = small.tile([P, 1], F32, tag="rln")
        w2s = small.tile([P, 1], F32, tag="w2s")
        nc.scalar.mul(out=mu_ln, in_=ln_ps[:, 0:1], mul=1.0 / C)
        nc.scalar.mul(out=m2_ln, in_=ln_ps[:, 1:2], mul=1.0 / C)
        # var_ln = m2 - mu^2
        nc.vector.tensor_mul(out=var_ln, in0=mu_ln, in1=mu_ln)
        nc.vector.tensor_sub(out=var_ln, in0=m2_ln, in1=var_ln)
        nc.scalar.activation(
            out=rln, in_=var_ln,
            func=mybir.ActivationFunctionType.Abs_reciprocal_sqrt,
            bias=epst, scale=1.0,
        )
        # w2s = -(r_ln * mu_ln)
        nc.vector.scalar_tensor_tensor(
            out=w2s, in0=rln, scalar=-1.0, in1=mu_ln,
            op0=mybir.AluOpType.mult, op1=mybir.AluOpType.mult,
        )
        # v_k = -(r_in * mu_in)
        for k in range(KT):
            nc.vector.scalar_tensor_tensor(
                out=vk[k], in0=rin[k], scalar=-1.0, in1=mv[k][:, 0:1],
                op0=mybir.AluOpType.mult, op1=mybir.AluOpType.mult,
            )

        # ---- A^T tiles: AT_k[i, o] = W1T*r_in[i] + r_ln*W2T ----
        at = [small.tile([P, C], mm_dt, tag=f"at{k}", name=f"at{k}") for k in range(KT)]
        for k in range(KT):
            tmp = small.tile([P, C], F32, tag=f"attmp{k}", name=f"attmp{k}")
            nc.vector.tensor_scalar_mul(out=tmp, in0=w2t[k], scalar1=rln)
            nc.vector.scalar_tensor_tensor(
                out=at[k], in0=w1t[k], scalar=rin[k], in1=tmp,
                op0=mybir.AluOpType.mult, op1=mybir.AluOpType.add,
            )

        # ---- bias and epilogue scalars per m ----
        gs = [pb[:, m : m + 1] for m in range(MT)]          # gamma_m
        bt = [pb[:, MT + m : MT + m + 1] for m in range(MT)]  # beta_m
        bs = [small.tile([P, 1], F32, tag=f"bs{m}", name=f"bs{m}") for m in range(MT)]
        for m in range(MT):
            bps = psum_su.tile([P, 1], F32, tag="setup")
            msl = slice(m * P, (m + 1) * P)
            nc.tensor.matmul(bps, w1t[0][:, msl], vk[0], start=True, stop=False)
            nc.tensor.matmul(bps, w1t[1][:, msl], vk[1], start=False, stop=False)
            nc.tensor.matmul(bps, w2t[0][:, msl], w2s, start=False, stop=False)
            nc.tensor.matmul(bps, w2t[1][:, msl], w2s, start=False, stop=True)
            # bs = gamma * bias + beta
            nc.scalar.activation(
                out=bs[m], in_=bps,
                func=mybir.ActivationFunctionType.Identity,
                scale=gs[m], bias=bt[m],
            )

        # ---- main matmul + fused epilogue + chunked DMA out ----
        at_mm = at
        for nb in range(NCH):
            for m in range(MT):
                stg = stage.tile([P, CHUNK], F32, tag=f"stage{m}", name=f"stage{m}")
                msl = slice(m * P, (m + 1) * P)
                pss = [psum_mm.tile([P, NQ], F32, tag="ps", name=f"ps{nb}_{m}_{q}") for q in range(QPC)]
                for k in range(KT):
                    for q in range(QPC):
                        qsl = slice(q * NQ, (q + 1) * NQ)
                        nc.tensor.matmul(
                            pss[q], at_mm[k][:, msl], xt[k][nb][:, qsl],
                            start=(k == 0), stop=(k == KT - 1),
                        )
                for q in range(QPC):
                    ps = pss[q]
                    qsl = slice(q * NQ, (q + 1) * NQ)
                    # epilogue: out = gamma*psum + (gamma*bias+beta)
                    if (nb * MT + m + (q if nb == 0 else 0)) % 2 == 0:
                        nc.scalar.activation(
                            out=stg[:, qsl], in_=ps,
                            func=mybir.ActivationFunctionType.Identity,
                            bias=bs[m], scale=gs[m],
                        )
                    else:
                        nc.vector.tensor_scalar(
                            out=stg[:, qsl], in0=ps, scalar1=gs[m],
                            scalar2=bs[m], op0=mybir.AluOpType.mult,
                            op1=mybir.AluOpType.add,
                        )
                if nb == 0:
                    for q in range(QPC):
                        nc.sync.dma_start(
                            out=out_r[m, :, nb * CHUNK + q * NQ : nb * CHUNK + (q + 1) * NQ],
                            in_=stg[:, q * NQ : (q + 1) * NQ],
                        )
                else:
                    nc.sync.dma_start(
                        out=out_r[m, :, nb * CHUNK : (nb + 1) * CHUNK], in_=stg
                    )

    nc.compile()
    return nc


_built = {}


def _get(use_f32r: bool = USE_F32R):
    if use_f32r not in _built:
        _built[use_f32r] = build(use_f32r)
    return _built[use_f32r]


def run(x, params, W, trace=False, use_f32r=USE_F32R, **kw):
    nc = _get(use_f32r)
    x = np.ascontiguousarray(np.asarray(x, dtype=np.float32))
    params = np.ascontiguousarray(np.asarray(params, dtype=np.float32))
    W = np.ascontiguousarray(np.asarray(W, dtype=np.float32))
    in_maps = [
        {
            "x": x[b].reshape(C, HW),
            "params": params[b],
            "W": W,
        }
        for b in range(B)
    ]
    res = run_bass_kernel_spmd(
        nc, in_maps, list(range(N_CORES)), trace=trace, **kw
    )
    out = np.stack(
        [res.results[b]["out"].reshape(C, H, W_SP) for b in range(B)]
    ).astype(np.float32)
    return out, res


def kernel(x, params, W):
    out, _ = run(x, params, W)
    return out


# revision 32
# speedup vs baseline: 1.0929x; 1.0381x over previous
"""Trainium2 Bass kernel for nn_Ada_PoLIN (InstanceNorm+LayerNorm -> concat ->
1x1 conv -> per-sample scale/shift).

Math: for sample b,
  IN = (x - mu_in) * r_in            (per-channel spatial stats)
  LN = (x - mu_ln) * r_ln            (per-sample stats)
  c  = W1 @ IN + W2 @ LN             (W = [W1 | W2], 1x1 conv)
  out = gamma * c + beta

This collapses to a single per-sample channel-mixing matmul:
  out[o, s] = gamma[o] * ( sum_i A[o,i] * x[i,s] + bias[o] ) + beta[o]
  A[o, i]   = W1[o,i] * r_in[i] + r_ln * W2[o,i]
  bias[o]   = -sum_i W1[o,i]*r_in[i]*mu_in[i] - r_ln*mu_ln*sum_i W2[o,i]

Sharding: data-parallel over batch, one sample per NeuronCore (B=8, 8 cores).
No cross-core communication. Per core: one pass over x for stats (bn_stats,
overlapped with DMA-in), build A^T (tiny), then a [256,256]x[256,16384]
matmul streamed through PSUM with the gamma/beta epilogue fused into the
PSUM->SBUF evacuation, and chunked DMA-out.
"""

import sys

if "/opt/trn_rl_repo" not in sys.path:
    sys.path.insert(0, "/opt/trn_rl_repo")

from contextlib import ExitStack

import numpy as np

import concourse.bacc as bacc
import concourse.tile as tile
from concourse import mybir
from concourse.bass_utils import run_bass_kernel_spmd
from concourse.masks import make_identity

B, C, H, W_SP = 8, 256, 128, 128
HW = H * W_SP            # 16384 spatial elements
TWO_C = 2 * C
N_CORES = 8
EPS = 1e-5
P = 128                  # partitions
KT = C // P              # 2 contraction (input-channel) tiles
MT = C // P              # 2 output-channel tiles
CHUNK = 2048             # spatial chunk per x tile / DMA
NCH = HW // CHUNK        # 8 chunks per k-tile
NSUB = CHUNK // 512      # bn_stats subgroups per chunk
NQ = 512                 # matmul free-dim chunk (one PSUM bank)
QPC = CHUNK // NQ        # matmul chunks per stage tile

USE_F32R = True          # float32r matmul: full-rate fp32 path on TensorE

F32 = mybir.dt.float32
F32R = mybir.dt.float32r


def build(use_f32r: bool = USE_F32R):
    nc = bacc.Bacc("TRN2", num_devices=N_CORES)
    x_ext = nc.declare_dram_parameter("x", [C, HW], F32, isOutput=False)
    p_ext = nc.declare_dram_parameter("params", [TWO_C], F32, isOutput=False)
    w_ext = nc.declare_dram_parameter("W", [C, TWO_C], F32, isOutput=False)
    out_ext = nc.declare_dram_parameter("out", [C, HW], F32, isOutput=True)

    x_r = x_ext.ap().rearrange("(t p) s -> t p s", p=P)      # [KT, 128, HW]
    out_r = out_ext.ap().rearrange("(t p) s -> t p s", p=P)  # [MT, 128, HW]
    p_r = p_ext.ap().rearrange("(g p) -> g p", p=P)          # [4, 128]
    w_r = w_ext.ap().rearrange("(t p) i -> t p i", p=P)      # [MT, 128, 2C]

    mm_dt = F32R if use_f32r else F32

    with tile.TileContext(nc) as tc, ExitStack() as ctx:
        xpool = ctx.enter_context(tc.tile_pool(name="x", bufs=1))
        wpool = ctx.enter_context(tc.tile_pool(name="w", bufs=1))
        small = ctx.enter_context(tc.tile_pool(name="small", bufs=1))
        stage = ctx.enter_context(tc.tile_pool(name="stage", bufs=4))
        psum_mm = ctx.enter_context(
            tc.tile_pool(name="psum_mm", bufs=6, space="PSUM")
        )
        psum_su = ctx.enter_context(
            tc.tile_pool(name="psum_su", bufs=2, space="PSUM")
        )

        # ---- constants / weights (ACT-ring DMAs, emitted first so the
        # PE transposes + ACT copies clear before stats work floods ACT) ----
        ident = small.tile([P, P], F32, tag="ident")
        make_identity(nc, ident)
        ones = small.tile([P, P], F32, tag="ones")
        nc.vector.memset(ones, 1.0)
        epst = small.tile([P, 1], F32, tag="eps")
        nc.vector.memset(epst, EPS)

        w_sb = [wpool.tile([P, TWO_C], F32, tag=f"wsb{m}", name=f"wsb{m}") for m in range(MT)]
        pg = small.tile([4, P], F32, tag="pg")

        def emit_w_dmas():
            # queued on the sync ring behind the c0 x chunks; MUST be emitted
            # before any reader of w_sb/pg (Tile deps follow emission order)
            for m_ in range(MT):
                nc.sync.dma_start(out=w_sb[m_], in_=w_r[m_])
            nc.sync.dma_start(out=pg, in_=p_r)

        # params transpose + W1T/W2T transposes, emitted after the W DMAs
        pb = small.tile([P, 4], F32, tag="pb")
        w1t = [small.tile([P, C], F32, tag=f"w1t{k}", name=f"w1t{k}") for k in range(KT)]
        w2t = [small.tile([P, C], F32, tag=f"w2t{k}", name=f"w2t{k}") for k in range(KT)]

        def emit_w_derived():
            pt_ps = psum_su.tile([P, 4], F32, tag="setup", name="pt_ps")
            nc.tensor.transpose(pt_ps, pg, ident[:4, :4])
            nc.scalar.copy(out=pb, in_=pt_ps)
            for k_ in range(KT):
                for m_ in range(MT):
                    ps_ = psum_su.tile([P, P], F32, tag="setup", name="tps")
                    nc.tensor.transpose(
                        ps_, w_sb[m_][:, k_ * P : (k_ + 1) * P], ident
                    )
                    nc.scalar.copy(out=w1t[k_][:, m_ * P : (m_ + 1) * P], in_=ps_)
                    ps2_ = psum_su.tile([P, P], F32, tag="setup", name="tps2")
                    nc.tensor.transpose(
                        ps2_, w_sb[m_][:, C + k_ * P : C + (k_ + 1) * P], ident
                    )
                    nc.scalar.copy(out=w2t[k_][:, m_ * P : (m_ + 1) * P], in_=ps2_)

        # ---- x load + one-pass stats: bn_stats on DVE for most chunks;
        # chunk c==ACT_C goes to ACT (sum + sumsq accum passes) to keep the
        # DVE backlog shorter than the DMA window; the last chunk is DMA'd
        # in two halves so its stats clear quickly after arrival ----
        ACT_C = -1
        DVE_N = NCH * NSUB  # bn_stats subgroup slots per k
        xt = [[None] * NCH for _ in range(KT)]
        st = [small.tile([P, DVE_N, 6], F32, tag=f"st{k}", name=f"st{k}") for k in range(KT)]
        slot = [0] * KT
        for c in range(NCH):
            for k in range(KT):
                t = xpool.tile([P, CHUNK], mm_dt, tag=f"x{k}_{c}", name=f"x{k}_{c}")
                xt[k][c] = t
                src_ap = x_r[k, :, c * CHUNK : (c + 1) * CHUNK]
                if use_f32r:
                    src_ap = src_ap.bitcast(mm_dt)
                if c == NCH - 1:
                    half = CHUNK // 2
                    nc.sync.dma_start(out=t[:, :half], in_=src_ap[:, :half])
                    nc.sync.dma_start(out=t[:, half:], in_=src_ap[:, half:])
                else:
                    nc.sync.dma_start(out=t, in_=src_ap)
                tf = t.bitcast(F32)
                tv = tf.rearrange("p (a b) -> p a b", b=512)
                for j in range(NSUB):
                    nc.vector.bn_stats(
                        out=st[k][:, slot[k], :], in_=tv[:, j, :]
                    )
                    slot[k] += 1
            if c == 0:
                emit_w_dmas()
                emit_w_derived()
            if c >= NCH - 2:
                # dense warm-up matmuls on the last chunk arrivals: bring the
                # PE clock-gate to 8/8 right before the real matmuls start
                for k in range(KT):
                    for q in range(2):
                        wps = psum_su.tile(
                            [P, NQ], F32, tag="setup", name=f"warm{c}_{k}_{q}"
                        )
                        nc.tensor.matmul(
                            wps, w1t[0][:, 0:P],
                            xt[k][c].bitcast(F32)[:, q * NQ : (q + 1) * NQ],
                            start=True, stop=True,
                        )

        assert slot[0] == DVE_N and slot[1] == DVE_N

        # ---- finalize stats ----
        mv = [small.tile([P, 2], F32, tag=f"mv{k}", name=f"mv{k}") for k in range(KT)]
        attmp = [small.tile([P, C], F32, tag=f"attmp{k}", name=f"attmp{k}") for k in range(KT)]
        rin = [small.tile([P, 1], F32, tag=f"rin{k}", name=f"rin{k}") for k in range(KT)]
        tk = [small.tile([P, 2], F32, tag=f"tk{k}", name=f"tk{k}") for k in range(KT)]
        vk = [small.tile([P, 1], F32, tag=f"vk{k}", name=f"vk{k}") for k in range(KT)]
        for k in range(KT):
            nc.vector.bn_aggr(out=mv[k], in_=st[k])
            mu_k = mv[k][:, 0:1]
            var_k = mv[k][:, 1:2]
            # r_in = 1/sqrt(var+eps)
            nc.scalar.activation(
                out=rin[k], in_=var_k,
                func=mybir.ActivationFunctionType.Abs_reciprocal_sqrt,
                bias=epst, scale=1.0,
            )
            nc.vector.tensor_scalar_mul(
                out=attmp[k], in0=w1t[k], scalar1=rin[k]
            )
            # tk = [mu, E[x^2]] for the LN cross-channel sums
            nc.vector.tensor_copy(out=tk[k][:, 0:1], in_=mu_k)
            nc.vector.scalar_tensor_tensor(
                out=tk[k][:, 1:2], in0=mu_k, scalar=mu_k, in1=var_k,
                op0=mybir.AluOpType.mult, op1=mybir.AluOpType.add,
            )

        # LN sums replicated on all partitions: ones^T @ t
        ln_ps = psum_su.tile([P, 2], F32, tag="setup")
        for k in range(KT):
            nc.tensor.matmul(
                ln_ps, ones, tk[k], start=(k == 0), stop=(k == KT - 1)
            )
        var_ln = small.tile([P, 1], F32, tag="var_ln")
        rln = small.tile([P, 1], F32, tag="rln")
        w2s = small.tile([P, 1], F32, tag="w2s")
        lnm = small.tile([P, 2], F32, tag="lnm")
        nc.vector.tensor_scalar_mul(out=lnm, in0=ln_ps, scalar1=1.0 / C)
        mu_ln = lnm[:, 0:1]
        m2_ln = lnm[:, 1:2]
        # var_ln = m2 - mu^2
        nc.vector.tensor_mul(out=var_ln, in0=mu_ln, in1=mu_ln)
        nc.vector.tensor_sub(out=var_ln, in0=m2_ln, in1=var_ln)
        nc.scalar.activation(
            out=rln, in_=var_ln,
            func=mybir.ActivationFunctionType.Abs_reciprocal_sqrt,
            bias=epst, scale=1.0,
        )
        # w2s = -(r_ln * mu_ln)
        nc.vector.scalar_tensor_tensor(
            out=w2s, in0=rln, scalar=-1.0, in1=mu_ln,
            op0=mybir.AluOpType.mult, op1=mybir.AluOpType.mult,
        )
        # v_k = -(r_in * mu_in)
        for k in range(KT):
            nc.vector.scalar_tensor_tensor(
                out=vk[k], in0=rin[k], scalar=-1.0, in1=mv[k][:, 0:1],
                op0=mybir.AluOpType.mult, op1=mybir.AluOpType.mult,
            )

        # ---- A^T tiles: AT_k[i, o] = W1T*r_in[i] + r_ln*W2T ----
        at = [small.tile([P, C], mm_dt, tag=f"at{k}", name=f"at{k}") for k in range(KT)]
        for k in range(KT):
            nc.vector.scalar_tensor_tensor(
                out=at[k], in0=w2t[k], scalar=rln, in1=attmp[k],
                op0=mybir.AluOpType.mult, op1=mybir.AluOpType.add,
            )

        # ---- bias and epilogue scalars per m (emitted inside the main
        # loop, after the first psum's matmuls, so the tiny bias matmuls
        # don't block the big ones in the PE queue) ----
        gs = [pb[:, m : m + 1] for m in range(MT)]          # gamma_m
        bt = [pb[:, MT + m : MT + m + 1] for m in range(MT)]  # beta_m
        bs = [small.tile([P, 1], F32, tag=f"bs{m}", name=f"bs{m}") for m in range(MT)]

        def emit_bias(m):
            bps = psum_su.tile([P, 1], F32, tag="setup", name=f"bps{m}")
            msl = slice(m * P, (m + 1) * P)
            nc.tensor.matmul(bps, w1t[0][:, msl], vk[0], start=True, stop=False)
            nc.tensor.matmul(bps, w1t[1][:, msl], vk[1], start=False, stop=False)
            nc.tensor.matmul(bps, w2t[0][:, msl], w2s, start=False, stop=False)
            nc.tensor.matmul(bps, w2t[1][:, msl], w2s, start=False, stop=True)
            # bs = gamma * bias + beta
            nc.scalar.activation(
                out=bs[m], in_=bps,
                func=mybir.ActivationFunctionType.Identity,
                scale=gs[m], bias=bt[m],
            )

        # ---- main matmul + fused epilogue + chunked DMA out ----
        at_mm = at
        for nb in range(NCH):
            for m in range(MT):
                stg = stage.tile([P, CHUNK], F32, tag=f"stage{m}", name=f"stage{m}")
                msl = slice(m * P, (m + 1) * P)
                for q in range(QPC):
                    ps = psum_mm.tile([P, NQ], F32)
                    qsl = slice(q * NQ, (q + 1) * NQ)
                    for k in range(KT):
                        rhs = xt[k][nb][:, qsl]
                        nc.tensor.matmul(
                            ps, at_mm[k][:, msl], rhs,
                            start=(k == 0), stop=(k == KT - 1),
                        )
                    if nb == 0 and q == 0:
                        emit_bias(m)
                    # epilogue: out = gamma*psum + (gamma*bias+beta)
                    if (nb * MT + m + (q if nb == 0 else 0)) % 2 == 0:
                        nc.scalar.activation(
                            out=stg[:, qsl], in_=ps,
                            func=mybir.ActivationFunctionType.Identity,
                            bias=bs[m], scale=gs[m],
                        )
                    else:
                        nc.vector.tensor_scalar(
                            out=stg[:, qsl], in0=ps, scalar1=gs[m],
                            scalar2=bs[m], op0=mybir.AluOpType.mult,
                            op1=mybir.AluOpType.add,
                        )
                if nb == 0:
                    for q in range(QPC):
                        nc.sync.dma_start(
                            out=out_r[m, :, nb * CHUNK + q * NQ : nb * CHUNK + (q + 1) * NQ],
                            in_=stg[:, q * NQ : (q + 1) * NQ],
                        )
                else:
                    nc.sync.dma_start(
                        out=out_r[m, :, nb * CHUNK : (nb + 1) * CHUNK], in_=stg
                    )

    nc.compile()
    return nc


_built = {}


def _get(use_f32r: bool = USE_F32R):
    if use_f32r not in _built:
        _built[use_f32r] = build(use_f32r)
    return _built[use_f32r]


def run(x, params, W, trace=False, use_f32r=USE_F32R, **kw):
    nc = _get(use_f32r)
    x = np.ascontiguousarray(np.asarray(x, dtype=np.float32))
    params = np.ascontiguousarray(np.asarray(params, dtype=np.float32))
    W = np.ascontiguousarray(np.asarray(W, dtype=np.float32))
    in_maps = [
        {
            "x": x[b].reshape(C, HW),
            "params": params[b],
            "W": W,
        }
        for b in range(B)
    ]
    res = run_bass_kernel_spmd(
        nc, in_maps, list(range(N_CORES)), trace=trace, **kw
    )
    out = np.stack(
        [res.results[b]["out"].reshape(C, H, W_SP) for b in range(B)]
    ).astype(np.float32)
    return out, res


def kernel(x, params, W):
    out, _ = run(x, params, W)
    return out
